# revision 37
# baseline (speedup 1.0000x reference)
"""BinaryConv2D Trainium2 kernel.

Reference op: out = conv2d(sign(clip(x,-1,1)), sign(clip(w,-1,1))),
NHWC x HWIO -> NHWC, SAME padding, stride 1, fp32.

sign() values are exactly representable in fp8e4 and every partial sum
is an integer bounded by 3*3*256 = 2304 (< 2^11), so the conv is EXACT
with fp8 DoubleRow matmuls (256-cin contraction per pass) into fp32
PSUM, and the output is exact in fp16.

Sharding: data-parallel over batch. 32 images / 8 cores = 4 per core;
weights replicated. No collectives.

Host prep (numpy, inside kernel()): binarize x and w to fp8 bytes
(+1 -> 0x38, -1 -> 0xB8, 0 -> 0x00), lay x out as a zero-padded pixel
stream with period 57 (56 cols + 1 shared pad slot; one extra leading
slot; top/bottom pad rows), and pack ADJACENT cin pairs into uint16 so
a 2-byte DMA transpose moves two fp8 lanes at once. Weights are
rearranged to the DoubleRow stationary layout [p, tap, ko, cout] with
(p, ko) <-> cin 2p+ko, matching the activation pairing.

Device per core:
  1. One DMA-transpose per image half (sync + scalar HWDGE queues):
     [1664 pix, 128 u16] DRAM -> SBUF [128, pix] -- contiguous source,
     so the fast xbar path applies. No staging, no casts, no sign.
  2. Conv as implicit GEMM, fp8 DoubleRow: psum[cout=128, 456]
     accumulates 9 taps; rhs is a contiguous 456-wide window of the
     padded stream viewed as [p, ko(stride 1B), pix(stride 2B)].
     1 junk column per 57 (the shared pad), dropped at evacuation.
  3. PSUM -> SBUF fp16 (strided DVE copy keeps 56 of 57 cols) -> DRAM
     out [2, 128, 12544] fp16 cout-major on SWDGE; host converts to
     fp32 NHWC while unsharding.
"""

import numpy as np
import ml_dtypes

import concourse.bass as bass
import concourse.mybir as mybir
from concourse import bacc
from concourse.tile import TileContext
from concourse.bass_utils import run_bass_kernel_spmd

F32 = mybir.dt.float32
F16 = mybir.dt.float16
BF16 = mybir.dt.bfloat16
FP8 = mybir.dt.float8e4

N_CORES = 8
N_IMG = 4            # images per core
H = W = 56
CIN = COUT = 256
NPIX = H * W                      # 3136 pixels per image
PW = W + 1                        # 57: padded stream row period
SAL = 3328                        # 1 + 58*57 = 3307, aligned up to 16
NT = 456                          # psum free size: 8 rows * 57
ROWBLK = 8
NBLK = H // ROWBLK                # 7
NTO = ROWBLK * W                  # 448 valid output pixels per tile


WROWS = 9 * COUT                  # 2304 weight rows prepended to x


def build(nc: bass.Bass):
    # x: [weights (2304 rows) | 4 padded fp8-pair image streams], one row
    # of 128 uint16 per slot. Weights are stored pre-transposed in the
    # DoubleRow stationary layout [p, tap, ko, cout] with (p,ko) = cin
    # 2p+ko, and ride the same xbar transpose path as the images (mixing
    # linear DMAs with transposes on HWDGE corrupts transfers); fusing
    # them into x lets ONE transpose land weights + image-0's first rows,
    # so the first matmul gates on a single completion semaphore.
    x_d = nc.dram_tensor("x", [WROWS + N_IMG * SAL, 128], BF16, kind="ExternalInput")
    y_d = nc.dram_tensor("y", [2, 128, N_IMG * NPIX], F16, kind="ExternalOutput")

    with TileContext(nc) as tc:
        with (
            tc.tile_pool(name="wpool", bufs=1) as wpool,
            tc.tile_pool(name="scr", bufs=1) as scrpool,
            tc.tile_pool(name="act", bufs=3) as actpool,
            tc.tile_pool(name="psum", bufs=7, space="PSUM") as psumpool,
            tc.tile_pool(name="wps", bufs=1, space="PSUM") as wpspool,
            tc.tile_pool(name="out", bufs=4) as outpool,
        ):
            # ALL transposes ride ONE HWDGE queue (scalar), strictly
            # serial: concurrent xbar DMAs on the two HWDGE queues
            # corrupt each other's first tiles (observed on HW). One DMA
            # per image keeps the total DMA count low -- semaphores are
            # a scarce rotating resource, and too many in-flight DMAs
            # couple unrelated streams through semaphore-reuse waits.
            def prep(n):
                """Transpose image n to channel-pair-major [128, SAL]."""
                t = actpool.tile([128, SAL], BF16, tag="act")
                nc.scalar.dma_start(
                    out=t[:],
                    in_=x_d[WROWS + n * SAL : WROWS + (n + 1) * SAL, :],
                    transpose=True,
                )
                return t[:]

            # warmup scratch: the PE clock gate (HAM) opens only after
            # ~3.4us of sustained PE activity; dummy matmuls during the
            # otherwise-idle head put the ramp before the real matmuls
            scr = scrpool.tile([128, 2, 256], FP8)
            nc.gpsimd.memset(scr[:], 0.0)

            # header tile = weights + image 0, filled by two transposes:
            # piece 1 (weights + image-0 rows covering the j=0 block)
            # gates the first matmul; piece 2 streams in behind it
            hdr = wpool.tile([128, WROWS + SAL], BF16)
            nc.scalar.dma_start(
                out=hdr[:, 0 : WROWS + 1024],
                in_=x_d[0 : WROWS + 1024, :],
                transpose=True,
            )
            nc.scalar.dma_start(
                out=hdr[:, WROWS + 1024 :],
                in_=x_d[WROWS + 1024 : WROWS + SAL, :],
                transpose=True,
            )
            wb8 = hdr[:, 0:WROWS].bitcast(FP8).rearrange(
                "p (t k c) -> p t k c", k=2, c=COUT
            )

            acts = {0: hdr[:, WROWS : WROWS + SAL]}

            # 17 full warmups (~3.6us cold) open the clock gate, then a
            # taper of small ones keeps the PE busy right up to the
            # first real matmul -- any idle gap re-arms the throttle
            wps = wpspool.tile([128, 256], F32)
            for _ in range(17):
                nc.tensor.matmul(
                    wps[:], scr[:, :, 0:128], scr[:],
                    start=True, stop=True,
                    perf_mode=mybir.MatmulPerfMode.DoubleRow,
                )
            for _ in range(14):
                nc.tensor.matmul(
                    wps[:, 0:96], scr[:, :, 0:128], scr[:, :, 0:96],
                    start=True, stop=True,
                    perf_mode=mybir.MatmulPerfMode.DoubleRow,
                )

            def rhs_ap(a8, j, t9):
                dy, dx = t9 // 3 - 1, t9 % 3 - 1
                base = 1 + PW * (ROWBLK * j + 1 + dy) + dx
                # 4D rhs AP [p, ko, row, col]: skips the 1-per-57 pad
                # columns entirely (FD 448)
                return a8[:, :, base : base + NT].rearrange(
                    "p k (r c) -> p k r c", c=PW
                )[:, :, :, 0:W]

            def evac(n, m, j, psum, ot):
                nc.vector.tensor_copy(ot[:, j * NTO : (j + 1) * NTO], psum[:])
                # One 802KB DMA per (image, cout half) keeps the DMA and
                # semaphore count low; the LAST image instead streams
                # per-block DMAs so the tail pipelines with the matmuls.
                # Images 2-3 ride the scalar queue (queue-ordered after
                # the last transpose, so no linear/xbar overlap) so the
                # slow SWDGE drain (images 0-1) hides under the matmuls.
                if n == N_IMG - 1:
                    nc.scalar.dma_start(
                        out=y_d[m][:, n * NPIX + j * NTO : n * NPIX + (j + 1) * NTO],
                        in_=ot[:, j * NTO : (j + 1) * NTO],
                    )
                elif j == NBLK - 1:
                    dma = nc.gpsimd.dma_start if n < 2 else nc.scalar.dma_start
                    dma(
                        out=y_d[m][:, n * NPIX : (n + 1) * NPIX], in_=ot[:]
                    )

            for n in range(N_IMG):
                if n + 1 < N_IMG:
                    acts[n + 1] = prep(n + 1)
                # [128, 2, SAL] view: ko stride 1 byte, pix stride 2 bytes
                a8 = acts[n].bitcast(FP8).rearrange("p (x k) -> p k x", k=2)
                for m in range(2):          # cout chunk
                    ot = outpool.tile([128, NPIX], F16, name="ot", tag="ot")
                    for j in range(NBLK):   # 8-row output block
                        psum = psumpool.tile([128, NTO], F32, name="ps", tag="ps")
                        for t9 in range(9):
                            nc.tensor.matmul(
                                psum[:],
                                wb8[:, t9, :, m * 128 : (m + 1) * 128],
                                rhs_ap(a8, j, t9),
                                start=(t9 == 0),
                                stop=(t9 == 8),
                                perf_mode=mybir.MatmulPerfMode.DoubleRow,
                            )
                        evac(n, m, j, psum, ot)
    return nc


def _sign_bytes(a: np.ndarray) -> np.ndarray:
    """fp8e4 encoding of sign(a): +1 -> 0x38, -1 -> 0xB8, 0 -> 0x00."""
    return np.where(a > 0, np.uint8(0x38), np.where(a < 0, np.uint8(0xB8), np.uint8(0))).astype(np.uint8)


def _prep_x(x: np.ndarray) -> np.ndarray:
    """(32,56,56,256) f32 -> (32, SAL, 128) uint16 padded fp8-pair stream."""
    s = _sign_bytes(x)                                   # (32,56,56,256) u8
    padded = np.zeros((32, 58, PW, CIN), np.uint8)       # rows: top pad, 56, bottom pad
    padded[:, 1:57, 0:W, :] = s
    stream = np.zeros((32, SAL, CIN), np.uint8)
    stream[:, 1 : 1 + 58 * PW, :] = padded.reshape(32, 58 * PW, CIN)
    return stream.view(np.uint16)                        # (32, SAL, 128)


def _prep_w(w: np.ndarray) -> np.ndarray:
    """(3,3,256,256) f32 -> (2304, 128) u16, the pre-transposed source whose
    DMA-transpose lands [p, tap, ko, cout] fp8 with cin = 2p+ko."""
    s = _sign_bytes(w).reshape(9, 128, 2, COUT)          # [t, p, ko, cout]
    flat = np.ascontiguousarray(s.transpose(1, 0, 2, 3)).reshape(128, 9 * 2 * COUT)
    return np.ascontiguousarray(flat.view(np.uint16).T)  # (2304, 128)


def _run(x: np.ndarray, w: np.ndarray, trace: bool = False, mode: str = "fp8"):
    """x: (32,56,56,256) f32, w: (3,3,256,256) f32 -> (out, BassKernelResults)."""
    nc = bacc.Bacc(None, target_bir_lowering=False, debug=False)
    build(nc)
    nc.finalize()
    xs_all = _prep_x(x)
    wf = _prep_w(w)
    in_maps = []
    for c in range(N_CORES):
        xs = np.concatenate(
            [wf, xs_all[c * N_IMG : (c + 1) * N_IMG].reshape(N_IMG * SAL, 128)]
        ).view(ml_dtypes.bfloat16)
        in_maps.append({"x": xs})
    res = run_bass_kernel_spmd(nc, in_maps, core_ids=list(range(N_CORES)), trace=trace)
    outs = []
    for c in range(N_CORES):
        y = np.asarray(res.results[c]["y"]).astype(np.float32)  # [2, 128, 12544]
        o = (
            y.reshape(2, 128, N_IMG, H, W)
            .transpose(2, 3, 4, 0, 1)
            .reshape(N_IMG, H, W, COUT)
        )
        outs.append(o)
    return np.concatenate(outs, axis=0).astype(np.float32), res


def kernel(**inputs) -> np.ndarray:
    x = np.asarray(inputs["inputs"], dtype=np.float32)
    w = np.asarray(inputs["kernel"], dtype=np.float32)
    out, _ = _run(x, w, trace=False)
    return out


# revision 38
# speedup vs baseline: 1.0008x; 1.0008x over previous
"""BinaryConv2D Trainium2 kernel.

Reference op: out = conv2d(sign(clip(x,-1,1)), sign(clip(w,-1,1))),
NHWC x HWIO -> NHWC, SAME padding, stride 1, fp32.

sign() values are exactly representable in fp8e4 and every partial sum
is an integer bounded by 3*3*256 = 2304 (< 2^11), so the conv is EXACT
with fp8 DoubleRow matmuls (256-cin contraction per pass) into fp32
PSUM, and the output is exact in fp16.

Sharding: data-parallel over batch. 32 images / 8 cores = 4 per core;
weights replicated. No collectives.

Host prep (numpy, inside kernel()): binarize x and w to fp8 bytes
(+1 -> 0x38, -1 -> 0xB8, 0 -> 0x00), lay x out as a zero-padded pixel
stream with period 57 (56 cols + 1 shared pad slot; one extra leading
slot; top/bottom pad rows), and pack ADJACENT cin pairs into uint16 so
a 2-byte DMA transpose moves two fp8 lanes at once. Weights are
rearranged to the DoubleRow stationary layout [p, tap, ko, cout] with
(p, ko) <-> cin 2p+ko (matching the activation pairing), pre-transposed
and PREPENDED to the x tensor so they ride the same xbar path.

Device per core (hard-won scheduling constraints in the inline
comments: single serialized transpose queue, DMA-count discipline,
HAM clock-gate warmup, completion-semaphore latency ~3us):
  1. One DMA-transpose per image: [3328 slots, 128 u16] DRAM -> SBUF
     [128 cin-pairs, 3328] -- contiguous source, fast xbar path. No
     staging, no casts, no sign ops on device.
  2. Conv as implicit GEMM, fp8 DoubleRow: psum[cout=128, 448]
     accumulates 9 taps; rhs is a 4D view [p, ko(1B), row(114B),
     col(2B)] of the padded stream, skipping pad columns (FD=448).
  3. PSUM -> SBUF fp16 (DVE copy) -> DRAM out [2, 128, 12544] fp16
     cout-major; host converts to fp32 NHWC while unsharding.
"""

import numpy as np
import ml_dtypes

import concourse.bass as bass
import concourse.mybir as mybir
from concourse import bacc
from concourse.tile import TileContext
from concourse.bass_utils import run_bass_kernel_spmd

F32 = mybir.dt.float32
F16 = mybir.dt.float16
BF16 = mybir.dt.bfloat16
FP8 = mybir.dt.float8e4

N_CORES = 8
N_IMG = 4            # images per core
H = W = 56
CIN = COUT = 256
NPIX = H * W                      # 3136 pixels per image
PW = W + 1                        # 57: padded stream row period
SAL = 3328                        # 1 + 58*57 = 3307, aligned up to 16
NT = 456                          # psum free size: 8 rows * 57
ROWBLK = 8
NBLK = H // ROWBLK                # 7
NTO = ROWBLK * W                  # 448 valid output pixels per tile


WROWS = 9 * COUT                  # 2304 weight rows prepended to x


def build(nc: bass.Bass):
    # x: [weights (2304 rows) | 4 padded fp8-pair image streams], one row
    # of 128 uint16 per slot. Weights are stored pre-transposed in the
    # DoubleRow stationary layout [p, tap, ko, cout] with (p,ko) = cin
    # 2p+ko, and ride the same xbar transpose path as the images (mixing
    # linear DMAs with transposes on HWDGE corrupts transfers); fusing
    # them into x lets ONE transpose land weights + image-0's first rows,
    # so the first matmul gates on a single completion semaphore.
    x_d = nc.dram_tensor("x", [WROWS + N_IMG * SAL, 128], BF16, kind="ExternalInput")
    y_d = nc.dram_tensor("y", [2, 128, N_IMG * NPIX], F16, kind="ExternalOutput")

    with TileContext(nc) as tc:
        with (
            tc.tile_pool(name="wpool", bufs=1) as wpool,
            tc.tile_pool(name="scr", bufs=1) as scrpool,
            tc.tile_pool(name="act", bufs=3) as actpool,
            tc.tile_pool(name="psum", bufs=7, space="PSUM") as psumpool,
            tc.tile_pool(name="wps", bufs=1, space="PSUM") as wpspool,
            tc.tile_pool(name="out", bufs=4) as outpool,
        ):
            # ALL transposes ride ONE HWDGE queue (scalar), strictly
            # serial: concurrent xbar DMAs on the two HWDGE queues
            # corrupt each other's first tiles (observed on HW). One DMA
            # per image keeps the total DMA count low -- semaphores are
            # a scarce rotating resource, and too many in-flight DMAs
            # couple unrelated streams through semaphore-reuse waits.
            def prep(n):
                """Transpose image n to channel-pair-major [128, SAL]."""
                t = actpool.tile([128, SAL], BF16, tag="act")
                nc.scalar.dma_start(
                    out=t[:],
                    in_=x_d[WROWS + n * SAL : WROWS + (n + 1) * SAL, :],
                    transpose=True,
                )
                return t[:]

            # warmup scratch: the PE clock gate (HAM) opens only after
            # ~3.4us of sustained PE activity; dummy matmuls during the
            # otherwise-idle head put the ramp before the real matmuls
            scr = scrpool.tile([128, 2, 256], FP8)
            nc.gpsimd.memset(scr[:], 0.0)

            # header tile = weights + image 0, filled by two transposes:
            # piece 1 (weights + image-0 rows covering the j=0 block)
            # gates the first matmul; piece 2 streams in behind it
            hdr = wpool.tile([128, WROWS + SAL], BF16)
            nc.scalar.dma_start(
                out=hdr[:, 0 : WROWS + 1024],
                in_=x_d[0 : WROWS + 1024, :],
                transpose=True,
            )
            nc.scalar.dma_start(
                out=hdr[:, WROWS + 1024 :],
                in_=x_d[WROWS + 1024 : WROWS + SAL, :],
                transpose=True,
            )
            wb8 = hdr[:, 0:WROWS].bitcast(FP8).rearrange(
                "p (t k c) -> p t k c", k=2, c=COUT
            )

            acts = {0: hdr[:, WROWS : WROWS + SAL]}

            # 17 full warmups (~3.6us cold) open the clock gate, then a
            # taper of small ones keeps the PE busy right up to the
            # first real matmul -- any idle gap re-arms the throttle
            wps = wpspool.tile([128, 256], F32)
            for _ in range(17):
                nc.tensor.matmul(
                    wps[:], scr[:, :, 0:128], scr[:],
                    start=True, stop=True,
                    perf_mode=mybir.MatmulPerfMode.DoubleRow,
                )
            for _ in range(14):
                nc.tensor.matmul(
                    wps[:, 0:96], scr[:, :, 0:128], scr[:, :, 0:96],
                    start=True, stop=True,
                    perf_mode=mybir.MatmulPerfMode.DoubleRow,
                )

            def rhs_ap(a8, j, t9):
                dy, dx = t9 // 3 - 1, t9 % 3 - 1
                base = 1 + PW * (ROWBLK * j + 1 + dy) + dx
                # 4D rhs AP [p, ko, row, col]: skips the 1-per-57 pad
                # columns entirely (FD 448)
                return a8[:, :, base : base + NT].rearrange(
                    "p k (r c) -> p k r c", c=PW
                )[:, :, :, 0:W]

            def evac(n, m, j, psum, ot):
                nc.vector.tensor_copy(ot[:, j * NTO : (j + 1) * NTO], psum[:])
                # One 802KB DMA per (image, cout half) keeps the DMA and
                # semaphore count low; the LAST image instead streams
                # per-block DMAs so the tail pipelines with the matmuls.
                # Images 2-3 ride the scalar queue (queue-ordered after
                # the last transpose, so no linear/xbar overlap) so the
                # slow SWDGE drain (images 0-1) hides under the matmuls.
                if n == N_IMG - 1:
                    nc.scalar.dma_start(
                        out=y_d[m][:, n * NPIX + j * NTO : n * NPIX + (j + 1) * NTO],
                        in_=ot[:, j * NTO : (j + 1) * NTO],
                    )
                elif j == NBLK - 1:
                    dma = nc.gpsimd.dma_start if n < 2 else nc.scalar.dma_start
                    dma(
                        out=y_d[m][:, n * NPIX : (n + 1) * NPIX], in_=ot[:]
                    )

            for n in range(N_IMG):
                if n + 1 < N_IMG:
                    acts[n + 1] = prep(n + 1)
                # [128, 2, SAL] view: ko stride 1 byte, pix stride 2 bytes
                a8 = acts[n].bitcast(FP8).rearrange("p (x k) -> p k x", k=2)
                for m in range(2):          # cout chunk
                    ot = outpool.tile([128, NPIX], F16, name="ot", tag="ot")
                    for j in range(NBLK):   # 8-row output block
                        psum = psumpool.tile([128, NTO], F32, name="ps", tag="ps")
                        for t9 in range(9):
                            nc.tensor.matmul(
                                psum[:],
                                wb8[:, t9, :, m * 128 : (m + 1) * 128],
                                rhs_ap(a8, j, t9),
                                start=(t9 == 0),
                                stop=(t9 == 8),
                                perf_mode=mybir.MatmulPerfMode.DoubleRow,
                            )
                        evac(n, m, j, psum, ot)
    return nc


def _sign_bytes(a: np.ndarray) -> np.ndarray:
    """fp8e4 encoding of sign(a): +1 -> 0x38, -1 -> 0xB8, 0 -> 0x00."""
    return np.where(a > 0, np.uint8(0x38), np.where(a < 0, np.uint8(0xB8), np.uint8(0))).astype(np.uint8)


def _prep_x(x: np.ndarray) -> np.ndarray:
    """(32,56,56,256) f32 -> (32, SAL, 128) uint16 padded fp8-pair stream."""
    s = _sign_bytes(x)                                   # (32,56,56,256) u8
    padded = np.zeros((32, 58, PW, CIN), np.uint8)       # rows: top pad, 56, bottom pad
    padded[:, 1:57, 0:W, :] = s
    stream = np.zeros((32, SAL, CIN), np.uint8)
    stream[:, 1 : 1 + 58 * PW, :] = padded.reshape(32, 58 * PW, CIN)
    return stream.view(np.uint16)                        # (32, SAL, 128)


def _prep_w(w: np.ndarray) -> np.ndarray:
    """(3,3,256,256) f32 -> (2304, 128) u16, the pre-transposed source whose
    DMA-transpose lands [p, tap, ko, cout] fp8 with cin = 2p+ko."""
    s = _sign_bytes(w).reshape(9, 128, 2, COUT)          # [t, p, ko, cout]
    flat = np.ascontiguousarray(s.transpose(1, 0, 2, 3)).reshape(128, 9 * 2 * COUT)
    return np.ascontiguousarray(flat.view(np.uint16).T)  # (2304, 128)


def _run(x: np.ndarray, w: np.ndarray, trace: bool = False, mode: str = "fp8"):
    """x: (32,56,56,256) f32, w: (3,3,256,256) f32 -> (out, BassKernelResults)."""
    nc = bacc.Bacc(None, target_bir_lowering=False, debug=False)
    build(nc)
    nc.finalize()
    xs_all = _prep_x(x)
    wf = _prep_w(w)
    in_maps = []
    for c in range(N_CORES):
        xs = np.concatenate(
            [wf, xs_all[c * N_IMG : (c + 1) * N_IMG].reshape(N_IMG * SAL, 128)]
        ).view(ml_dtypes.bfloat16)
        in_maps.append({"x": xs})
    res = run_bass_kernel_spmd(nc, in_maps, core_ids=list(range(N_CORES)), trace=trace)
    outs = []
    for c in range(N_CORES):
        y = np.asarray(res.results[c]["y"]).astype(np.float32)  # [2, 128, 12544]
        o = (
            y.reshape(2, 128, N_IMG, H, W)
            .transpose(2, 3, 4, 0, 1)
            .reshape(N_IMG, H, W, COUT)
        )
        outs.append(o)
    return np.concatenate(outs, axis=0).astype(np.float32), res


def kernel(**inputs) -> np.ndarray:
    x = np.asarray(inputs["inputs"], dtype=np.float32)
    w = np.asarray(inputs["kernel"], dtype=np.float32)
    out, _ = _run(x, w, trace=False)
    return out


# revision 39
# speedup vs baseline: 1.0094x; 1.0086x over previous
"""BinaryConv2D Trainium2 kernel.

Reference op: out = conv2d(sign(clip(x,-1,1)), sign(clip(w,-1,1))),
NHWC x HWIO -> NHWC, SAME padding, stride 1, fp32.

sign() values are exactly representable in fp8e4 and every partial sum
is an integer bounded by 3*3*256 = 2304 (< 2^11), so the conv is EXACT
with fp8 DoubleRow matmuls (256-cin contraction per pass) into fp32
PSUM, and the output is exact in fp16.

Sharding: data-parallel over batch. 32 images / 8 cores = 4 per core;
weights replicated. No collectives.

Host prep (numpy, inside kernel()): binarize x and w to fp8 bytes
(+1 -> 0x38, -1 -> 0xB8, 0 -> 0x00), lay x out as a zero-padded pixel
stream with period 57 (56 cols + 1 shared pad slot; one extra leading
slot; top/bottom pad rows), and pack ADJACENT cin pairs into uint16 so
a 2-byte DMA transpose moves two fp8 lanes at once. Weights are
rearranged to the DoubleRow stationary layout [p, tap, ko, cout] with
(p, ko) <-> cin 2p+ko (matching the activation pairing), pre-transposed
and PREPENDED to the x tensor so they ride the same xbar path.

Device per core (hard-won scheduling constraints in the inline
comments: single serialized transpose queue, DMA-count discipline,
HAM clock-gate warmup, completion-semaphore latency ~3us):
  1. One DMA-transpose per image: [3328 slots, 128 u16] DRAM -> SBUF
     [128 cin-pairs, 3328] -- contiguous source, fast xbar path. No
     staging, no casts, no sign ops on device.
  2. Conv as implicit GEMM, fp8 DoubleRow: psum[cout=128, 448]
     accumulates 9 taps; rhs is a 4D view [p, ko(1B), row(114B),
     col(2B)] of the padded stream, skipping pad columns (FD=448).
  3. PSUM -> SBUF fp16 (DVE copy) -> DRAM out [2, 128, 12544] fp16
     cout-major; host converts to fp32 NHWC while unsharding.
"""

import numpy as np
import ml_dtypes

import concourse.bass as bass
import concourse.mybir as mybir
from concourse import bacc
from concourse.tile import TileContext
from concourse.bass_utils import run_bass_kernel_spmd

F32 = mybir.dt.float32
F16 = mybir.dt.float16
BF16 = mybir.dt.bfloat16
FP8 = mybir.dt.float8e4

N_CORES = 8
N_IMG = 4            # images per core
H = W = 56
CIN = COUT = 256
NPIX = H * W                      # 3136 pixels per image
PW = W + 1                        # 57: padded stream row period
SAL = 3328                        # 1 + 58*57 = 3307, aligned up to 16
NT = 456                          # psum free size: 8 rows * 57
ROWBLK = 8
NBLK = H // ROWBLK                # 7
NTO = ROWBLK * W                  # 448 valid output pixels per tile


WROWS = 9 * COUT                  # 2304 weight rows prepended to x


def build(nc: bass.Bass):
    # x: [weights (2304 rows) | 4 padded fp8-pair image streams], one row
    # of 128 uint16 per slot. Weights are stored pre-transposed in the
    # DoubleRow stationary layout [p, tap, ko, cout] with (p,ko) = cin
    # 2p+ko, and ride the same xbar transpose path as the images (mixing
    # linear DMAs with transposes on HWDGE corrupts transfers); fusing
    # them into x lets ONE transpose land weights + image-0's first rows,
    # so the first matmul gates on a single completion semaphore.
    x_d = nc.dram_tensor("x", [N_IMG * SAL, 128], BF16, kind="ExternalInput")
    w_d = nc.dram_tensor("w", [WROWS, 128], BF16, kind="ExternalInput")
    y_d = nc.dram_tensor("y", [2, 128, N_IMG * NPIX], F16, kind="ExternalOutput")

    with TileContext(nc) as tc:
        with (
            tc.tile_pool(name="wpool", bufs=1) as wpool,
            tc.tile_pool(name="scr", bufs=1) as scrpool,
            tc.tile_pool(name="act", bufs=3) as actpool,
            tc.tile_pool(name="psum", bufs=7, space="PSUM") as psumpool,
            tc.tile_pool(name="wps", bufs=1, space="PSUM") as wpspool,
            tc.tile_pool(name="out", bufs=4) as outpool,
        ):
            # ALL transposes ride ONE HWDGE queue (scalar), strictly
            # serial: concurrent xbar DMAs on the two HWDGE queues
            # corrupt each other's first tiles (observed on HW). One DMA
            # per image keeps the total DMA count low -- semaphores are
            # a scarce rotating resource, and too many in-flight DMAs
            # couple unrelated streams through semaphore-reuse waits.
            def prep(n):
                """Transpose image n to channel-pair-major [128, SAL]."""
                t = actpool.tile([128, SAL], BF16, tag="act")
                nc.scalar.dma_start(
                    out=t[:],
                    in_=x_d[n * SAL : (n + 1) * SAL, :],
                    transpose=True,
                )
                return t[:]

            # warmup scratch: the PE clock gate (HAM) opens only after
            # ~3.4us of sustained PE activity; dummy matmuls during the
            # otherwise-idle head put the ramp before the real matmuls
            scr = scrpool.tile([128, 2, 256], FP8)
            nc.gpsimd.memset(scr[:], 0.0)

            # head pieces split small and ordered by first use (taps
            # 0-5, image-0 rows covering the j=0 block, taps 6-8, rest):
            # completion semaphores fire ~3us after the slice, so the
            # gating pieces must end their slices as early as possible
            wb16 = wpool.tile([128, WROWS], BF16)
            act0 = actpool.tile([128, SAL], BF16, tag="act")
            nc.scalar.dma_start(
                out=wb16[:, 0 : 6 * COUT], in_=w_d[0 : 6 * COUT, :], transpose=True
            )
            nc.scalar.dma_start(
                out=act0[:, 0:1024], in_=x_d[0:1024, :], transpose=True
            )
            nc.scalar.dma_start(
                out=wb16[:, 6 * COUT :], in_=w_d[6 * COUT :, :], transpose=True
            )
            nc.scalar.dma_start(
                out=act0[:, 1024:SAL], in_=x_d[1024:SAL, :], transpose=True
            )
            wb8 = wb16[:].bitcast(FP8).rearrange(
                "p (t k c) -> p t k c", k=2, c=COUT
            )

            acts = {0: act0[:]}

            # 17 full warmups (~3.6us cold) open the clock gate, then a
            # taper of small ones keeps the PE busy right up to the
            # first real matmul -- any idle gap re-arms the throttle
            wps = wpspool.tile([128, 256], F32)
            for _ in range(17):
                nc.tensor.matmul(
                    wps[:], scr[:, :, 0:128], scr[:],
                    start=True, stop=True,
                    perf_mode=mybir.MatmulPerfMode.DoubleRow,
                )
            for _ in range(14):
                nc.tensor.matmul(
                    wps[:, 0:96], scr[:, :, 0:128], scr[:, :, 0:96],
                    start=True, stop=True,
                    perf_mode=mybir.MatmulPerfMode.DoubleRow,
                )

            def rhs_ap(a8, j, t9):
                dy, dx = t9 // 3 - 1, t9 % 3 - 1
                base = 1 + PW * (ROWBLK * j + 1 + dy) + dx
                # 4D rhs AP [p, ko, row, col]: skips the 1-per-57 pad
                # columns entirely (FD 448)
                return a8[:, :, base : base + NT].rearrange(
                    "p k (r c) -> p k r c", c=PW
                )[:, :, :, 0:W]

            def evac(n, m, j, psum, ot):
                nc.vector.tensor_copy(ot[:, j * NTO : (j + 1) * NTO], psum[:])
                # One 802KB DMA per (image, cout half) keeps the DMA and
                # semaphore count low; the LAST image instead streams
                # per-block DMAs so the tail pipelines with the matmuls.
                # Images 2-3 ride the scalar queue (queue-ordered after
                # the last transpose, so no linear/xbar overlap) so the
                # slow SWDGE drain (images 0-1) hides under the matmuls.
                if n == N_IMG - 1:
                    nc.scalar.dma_start(
                        out=y_d[m][:, n * NPIX + j * NTO : n * NPIX + (j + 1) * NTO],
                        in_=ot[:, j * NTO : (j + 1) * NTO],
                    )
                elif j == NBLK - 1:
                    dma = nc.gpsimd.dma_start if n < 2 else nc.scalar.dma_start
                    dma(
                        out=y_d[m][:, n * NPIX : (n + 1) * NPIX], in_=ot[:]
                    )

            for n in range(N_IMG):
                if n + 1 < N_IMG:
                    acts[n + 1] = prep(n + 1)
                # [128, 2, SAL] view: ko stride 1 byte, pix stride 2 bytes
                a8 = acts[n].bitcast(FP8).rearrange("p (x k) -> p k x", k=2)
                for m in range(2):          # cout chunk
                    ot = outpool.tile([128, NPIX], F16, name="ot", tag="ot")
                    for j in range(NBLK):   # 8-row output block
                        psum = psumpool.tile([128, NTO], F32, name="ps", tag="ps")
                        for t9 in range(9):
                            nc.tensor.matmul(
                                psum[:],
                                wb8[:, t9, :, m * 128 : (m + 1) * 128],
                                rhs_ap(a8, j, t9),
                                start=(t9 == 0),
                                stop=(t9 == 8),
                                perf_mode=mybir.MatmulPerfMode.DoubleRow,
                            )
                        evac(n, m, j, psum, ot)
    return nc


def _sign_bytes(a: np.ndarray) -> np.ndarray:
    """fp8e4 encoding of sign(a): +1 -> 0x38, -1 -> 0xB8, 0 -> 0x00."""
    return np.where(a > 0, np.uint8(0x38), np.where(a < 0, np.uint8(0xB8), np.uint8(0))).astype(np.uint8)


def _prep_x(x: np.ndarray) -> np.ndarray:
    """(32,56,56,256) f32 -> (32, SAL, 128) uint16 padded fp8-pair stream."""
    s = _sign_bytes(x)                                   # (32,56,56,256) u8
    padded = np.zeros((32, 58, PW, CIN), np.uint8)       # rows: top pad, 56, bottom pad
    padded[:, 1:57, 0:W, :] = s
    stream = np.zeros((32, SAL, CIN), np.uint8)
    stream[:, 1 : 1 + 58 * PW, :] = padded.reshape(32, 58 * PW, CIN)
    return stream.view(np.uint16)                        # (32, SAL, 128)


def _prep_w(w: np.ndarray) -> np.ndarray:
    """(3,3,256,256) f32 -> (2304, 128) u16, the pre-transposed source whose
    DMA-transpose lands [p, tap, ko, cout] fp8 with cin = 2p+ko."""
    s = _sign_bytes(w).reshape(9, 128, 2, COUT)          # [t, p, ko, cout]
    flat = np.ascontiguousarray(s.transpose(1, 0, 2, 3)).reshape(128, 9 * 2 * COUT)
    return np.ascontiguousarray(flat.view(np.uint16).T)  # (2304, 128)


def _run(x: np.ndarray, w: np.ndarray, trace: bool = False, mode: str = "fp8"):
    """x: (32,56,56,256) f32, w: (3,3,256,256) f32 -> (out, BassKernelResults)."""
    nc = bacc.Bacc(None, target_bir_lowering=False, debug=False)
    build(nc)
    nc.finalize()
    xs_all = _prep_x(x)
    wf = _prep_w(w).view(ml_dtypes.bfloat16)
    in_maps = []
    for c in range(N_CORES):
        xs = np.ascontiguousarray(
            xs_all[c * N_IMG : (c + 1) * N_IMG].reshape(N_IMG * SAL, 128)
        ).view(ml_dtypes.bfloat16)
        in_maps.append({"x": xs, "w": wf})
    res = run_bass_kernel_spmd(nc, in_maps, core_ids=list(range(N_CORES)), trace=trace)
    outs = []
    for c in range(N_CORES):
        y = np.asarray(res.results[c]["y"]).astype(np.float32)  # [2, 128, 12544]
        o = (
            y.reshape(2, 128, N_IMG, H, W)
            .transpose(2, 3, 4, 0, 1)
            .reshape(N_IMG, H, W, COUT)
        )
        outs.append(o)
    return np.concatenate(outs, axis=0).astype(np.float32), res


def kernel(**inputs) -> np.ndarray:
    x = np.asarray(inputs["inputs"], dtype=np.float32)
    w = np.asarray(inputs["kernel"], dtype=np.float32)
    out, _ = _run(x, w, trace=False)
    return out
